# revision 23
# baseline (speedup 1.0000x reference)
"""Trainium2 Bass kernel for nn_Encoder_28595892256995 (gnn_message_passing).

Data parallel: 8 cores x 32 batch. Per core the octree merge net runs fully
on-chip in a slot-packed feature-major layout [slots*odim partitions,
(bo, j) columns], slots = 128//odim (idim <= odim at every level).

Host prep (data independent):
  * Tree relabeling perms p_t (p_{t-1}[2j] = cl_t[p_t[j]], ...[2j+1] = cr)
    -> children of level-t node j are adjacent columns 2j, 2j+1 of level
    t-1's stored output: tree gathers become stride-2 APs.
  * drev/dmap folded into per-direction effective weights (row swap).
  * "border" chain: batch->slot assignment; when slot count halves, each
    slot's ordered batch list is split into contiguous chunks, so every
    cross-level read touches one contiguous partition range.
  * Block-diagonal weight materialization for slot packing.
Host prep (input dependent, O(B*M) int ops = one-hot of direction ids):
  * int8 select masks in the device layout, streamed and consumed by
    copy_predicated to pick among the 6 per-expert PSUM banks.

Device per level: 6 all-expert block-diag matmul groups into 6 PSUM banks,
ACT evicts bank 0, DVE copy_predicated overwrites with banks 1..5 by mask,
ACT prelu. Sampled layers add an indirect_copy skip gather + Ws/Wms matmuls.
"""

import sys
import types
import numpy as np

NDIR = 6
SL = 3
B = 256
N = 4096
SPECS = [(2048, 8, 16, None), (1024, 16, 16, None), (512, 16, 32, 8), (256, 32, 32, 16),
         (128, 32, 64, 16), (64, 64, 64, 32), (32, 64, 128, 32), (16, 128, 128, 64),
         (8, 128, 256, 64), (4, 256, 256, 128), (2, 256, 512, 128), (1, 512, 512, 256)]
NL = len(SPECS)
NCORES = 8
BC = B // NCORES


def _np(x):
    return np.asarray(x)


# ---------------------------------------------------------------------------
# layout metadata
# ---------------------------------------------------------------------------

def _meta():
    m = []
    for t, (M, idim, odim, sd) in enumerate(SPECS):
        s = max(1, 128 // odim)
        och = max(1, odim // 128)
        m.append(dict(M=M, idim=idim, odim=odim, sd=sd, s=s, nb=BC // s,
                      och=och, cols=M * (BC // s)))
    return m


META = _meta()
S0 = 16                    # leaf slots (L1 reads with halve=2)
NB0 = BC // S0
LEAF_COLS = N * NB0        # 8192


def _borders():
    """border[t][slot] = ordered batch list; t = -1 (leaf) .. 11."""
    border = {11: [list(range(BC))]}
    for t in range(10, -2, -1):
        s_here = META[t]['s'] if t >= 0 else S0
        nxt = border[t + 1]
        s_next = len(nxt)
        if s_here == s_next:
            border[t] = [list(x) for x in nxt]
        else:
            halve = s_here // s_next
            pnb = len(nxt[0]) // halve
            cur = [None] * s_here
            for st in range(s_next):
                for q in range(halve):
                    cur[q * s_next + st] = list(nxt[st][q * pnb:(q + 1) * pnb])
            border[t] = cur
    return border


BORDER = _borders()


def _compute_perms(tree):
    p = [None] * (NL + 1)          # p[t+1] = slot->orig node for level t; p[0] leaf
    p[NL] = np.zeros(1, np.int64)
    for t in range(NL - 1, -1, -1):
        cl = _np(tree[t]['cl']).astype(np.int64)
        cr = _np(tree[t]['cr']).astype(np.int64)
        pt = p[t + 1]
        prev = np.empty(2 * len(pt), np.int64)
        prev[0::2] = cl[pt]
        prev[1::2] = cr[pt]
        p[t] = prev
    return p


def _blockdiag(w, s):
    k, m = w.shape
    out = np.zeros((s * k, s * m), np.float32)
    for i in range(s):
        out[i * k:(i + 1) * k, i * m:(i + 1) * m] = w
    return out


# ---------------------------------------------------------------------------
# main entry
# ---------------------------------------------------------------------------

def kernel(**inputs):
    points = _np(inputs['points']).astype(np.float32)
    vecs = [_np(inputs[f'vec{t}']).astype(np.int64) for t in range(1, 13)]
    params = inputs['params']
    tree = inputs['tree']
    drev = _np(inputs['drev']).astype(np.int64)
    dmap = _np(inputs['dmap']).astype(np.int64)
    levels = params['levels']
    leaf = params['leaf']

    perms = _compute_perms(tree)
    lw = _np(leaf['W']).astype(np.float32)
    lb = _np(leaf['b']).astype(np.float32)
    la = float(_np(leaf['a']))

    Weff, alphas = [], []
    ok = np.all(lb == 0.0)
    for t in range(NL):
        M, idim, odim, sd = SPECS[t]
        Wm = _np(levels[t]['Wm']).astype(np.float32)
        bm = _np(levels[t]['bm']).astype(np.float32)
        am = _np(levels[t]['am']).astype(np.float32)
        ok = ok and np.all(bm == 0.0) and np.all(am == am.flat[0])
        We = np.empty_like(Wm)
        for e in range(NDIR):
            w = Wm[dmap[e]]
            if drev[e]:
                w = np.concatenate([w[idim:], w[:idim]], axis=0)
            We[e] = w
        Weff.append(We)
        al = dict(am=float(am.flat[0]))
        if sd is not None:
            p = levels[t]
            ok = ok and np.all(_np(p['bs']) == 0.0) and np.all(_np(p['bms']) == 0.0)
            al['als'] = float(_np(p['als']))
            al['alms'] = float(_np(p['alms']))
        alphas.append(al)

    if not ok:
        return _host_reference(points, vecs, params, tree, drev, dmap)

    return _run_device(points, vecs, tree, perms, lw, la, Weff, alphas,
                       levels).astype(np.float32)


def _host_reference(points, vecs, params, tree, drev, dmap):
    def prelu(x, a):
        return np.where(x >= 0, x, a * x)
    lf = params['leaf']
    ans = prelu(points @ _np(lf['W']) + _np(lf['b']), float(_np(lf['a'])))
    backup = [ans]
    for p, tr, vec in zip(params['levels'], tree, vecs):
        cl = _np(tr['cl']).astype(np.int64)
        cr = _np(tr['cr']).astype(np.int64)
        lch = ans[:, cl]
        rch = ans[:, cr]
        dr = drev[vec].astype(ans.dtype)[:, :, None]
        x = np.concatenate([lch * (1 - dr) + rch * dr,
                            rch * (1 - dr) + lch * dr], axis=-1)
        v = dmap[vec]
        Wm = _np(p['Wm']); bm = _np(p['bm']); am = _np(p['am'])
        out = np.zeros(x.shape[:2] + (bm.shape[-1],), x.dtype)
        for i in range(NDIR):
            y = prelu(x @ Wm[i] + bm[i], am[i])
            out = np.where((v == i)[:, :, None], y, out)
        if 'Ws' in p:
            smp = prelu(backup[-SL][:, _np(tr['cs']).astype(np.int64)] @ _np(p['Ws'])
                        + _np(p['bs']), float(_np(p['als'])))
            out = prelu(np.concatenate([out, smp], axis=-1) @ _np(p['Wms'])
                        + _np(p['bms']), float(_np(p['alms'])))
        backup.append(out)
        ans = out
    return ans[:, 0]


# ---------------------------------------------------------------------------
# host-side data builders
# ---------------------------------------------------------------------------

def _src_pos_tables(t):
    """For sampled level t: map local batch b -> (src_slot, src_bo) in source
    level's border, plus target (st, bo) -> b."""
    ts = t - SL
    bsrc = BORDER[ts] if ts >= 0 else BORDER[-1]
    pos = {}
    for ss, lst in enumerate(bsrc):
        for i, b in enumerate(lst):
            pos[b] = (ss, i)
    return pos


def skip_bases(t):
    """Per contiguous bo-run (length src_nb) of sampled level t: the source
    slot-block base, with src_slot(b) == base + st verified for all slots."""
    m = META[t]
    ts = t - SL
    src_nb = META[ts]['nb'] if ts >= 0 else NB0
    pos = _src_pos_tables(t)
    s, nb = m['s'], m['nb']
    runs = nb // src_nb
    bases = []
    for ri in range(runs):
        base = pos[BORDER[t][0][ri * src_nb]][0]
        for st in range(s):
            for bo in range(ri * src_nb, (ri + 1) * src_nb):
                b = BORDER[t][st][bo]
                ss, sbo = pos[b]
                assert ss == base + st and sbo == bo % src_nb, \
                    (t, st, bo, ss, base, sbo)
        bases.append(base)
    return bases


def build_skip_idx(t, perms, tree):
    """uint16 idx [cols(+och folds)] for indirect_copy gather of skip features.
    Returns array [och_src * cols] (och_src = source level's ochunks)."""
    m = META[t]
    ts = t - SL
    if ts >= 0:
        src_M, src_cols = META[ts]['M'], META[ts]['cols']
        src_och = META[ts]['och']
        src_perm_inv = np.argsort(perms[ts + 1])
    else:
        src_M, src_cols, src_och = N, LEAF_COLS, 1
        src_perm_inv = np.argsort(perms[0])
    cs = _np(tree[t]['cs']).astype(np.int64)
    cs_slot = src_perm_inv[cs[perms[t + 1]]]          # [M_t]
    pos = _src_pos_tables(t)
    s, nb, M = m['s'], m['nb'], m['M']
    idx = np.empty((s, nb, M), np.int64)
    for st in range(s):
        for bo in range(nb):
            b = BORDER[t][st][bo]
            ss, sbo = pos[b]
            idx[st, bo, :] = sbo * src_M + cs_slot
    # all slots see the same column list? idx depends on st only through b ->
    # (ss, sbo); by border-chunk construction sbo == bo % src_nb for all st.
    base = idx[0]                                      # [nb, M]
    assert np.all(idx == base[None]), "skip idx must be slot independent"
    flat = base.reshape(-1)
    assert flat.max() + (src_och - 1) * src_cols < 65536
    return flat.astype(np.int64), src_och, src_cols


def build_core_arrays(core, points, vecs, perms):
    bsl = slice(core * BC, (core + 1) * BC)
    pc = points[bsl][:, perms[0], :]                   # [32, 4096, 3]
    leaf_rhs = np.empty((S0 * 3, LEAF_COLS), np.float32)
    for st in range(S0):
        for bo in range(NB0):
            b = BORDER[-1][st][bo]
            leaf_rhs[st * 3:(st + 1) * 3, bo * N:(bo + 1) * N] = pc[b].T
    arrs = {'leaf_rhs': np.ascontiguousarray(leaf_rhs)}
    for t in range(NL):
        m = META[t]
        s, nb, M, odim, och = m['s'], m['nb'], m['M'], m['odim'], m['och']
        vslot = vecs[t][bsl][:, perms[t + 1]]          # [32, M]
        v = np.empty((s, nb, M), np.int64)
        for st in range(s):
            for bo in range(nb):
                v[st, bo] = vslot[BORDER[t][st][bo]]
        # partition p = (st, o) (o in [0, min(odim,128)/...]) ; for och>1 the
        # mask repeats per oc block in the free dim.
        po = 128 // s                                   # = min(odim,128)
        vrep = np.repeat(v.reshape(s, 1, nb * M), po, axis=1).reshape(128, nb * M)
        if och > 1:
            vrep = np.tile(vrep, (1, 1))
            vrep = np.concatenate([vrep] * och, axis=1)  # [128, och*cols]
        masks = np.stack([(vrep == g) for g in range(1, NDIR)]).astype(np.int8)
        arrs[f'mask{t}'] = np.ascontiguousarray(masks)
    return arrs


# ---------------------------------------------------------------------------
# device program
# ---------------------------------------------------------------------------

def _install_ntff_hook():
    try:
        import antenv.axon_hooks  # noqa: F401
        return
    except ImportError:
        pass
    try:
        from trn_agent_boot.trn_boot import _ntff_profile_via_ctypes
        hook = _ntff_profile_via_ctypes('/opt/axon/libaxon_pjrt.so')
    except Exception:
        hook = None
    mod = types.ModuleType('antenv.axon_hooks')
    mod.get_axon_ntff_profile_hook = lambda: hook
    mod.set_axon_ntff_profile_hook = lambda h: None
    sys.modules['antenv.axon_hooks'] = mod


def _ap(tile_ap, plo, pcnt, free_dims, free_off):
    """Custom AP on a tile: partition range [plo, plo+pcnt), free pattern."""
    import dataclasses
    pstride = tile_ap.ap[0][0]
    return dataclasses.replace(
        tile_ap, ap=[[pstride, pcnt]] + [list(d) for d in free_dims],
        offset=tile_ap.offset + plo * pstride + free_off)


def build_program(Weff, alphas, levels, tree, perms, lw, la):
    _install_ntff_hook()
    import concourse.bacc as bacc
    import concourse.mybir as mybir
    from concourse import tile

    nc = bacc.Bacc("TRN2", target_bir_lowering=False, debug=False)
    f32 = mybir.dt.float32
    AF = mybir.ActivationFunctionType

    wconst = {}
    ext = {}

    def add_input(name, shape, dtype):
        ext[name] = nc.dram_tensor(name, list(shape), dtype, kind='ExternalInput')
        return ext[name]

    def add_const(name, arr):
        if arr.ndim == 2 and arr.shape[0] > 128:
            # fold contract chunks into the free dim: [K, M] -> [128, (K/128)*M]
            kf, M_ = arr.shape[0] // 128, arr.shape[1]
            arr = arr.reshape(kf, 128, M_).transpose(1, 0, 2).reshape(128, kf * M_)
        arr = np.ascontiguousarray(arr)
        wconst[name] = arr
        dt = {np.dtype(np.float32): f32, np.dtype(np.uint16): mybir.dt.uint16,
              np.dtype(np.int8): mybir.dt.int8}[arr.dtype]
        return add_input(name, arr.shape, dt)

    add_input('leaf_rhs', [S0 * 3, LEAF_COLS], f32)
    for t in range(NL):
        add_input(f'mask{t}', [5, 128, META[t]['och'] * META[t]['cols']], mybir.dt.int8)
    y_out = nc.dram_tensor('y', [BC, SPECS[-1][2]], f32, kind='ExternalOutput')
    import os
    dbg = {}
    if os.environ.get('KERNEL_DEBUG'):
        dbg[-1] = nc.dram_tensor('dbg_leaf', [S0 * 8, LEAF_COLS], f32,
                                 kind='ExternalOutput')
        for t in range(NL):
            dbg[t] = nc.dram_tensor(f'dbg{t}', [128, META[t]['och'] * META[t]['cols']],
                                    f32, kind='ExternalOutput')

    add_const('lw_bd', _blockdiag(lw, S0))
    for t in range(NL):
        m = META[t]
        s, idim, odim = m['s'], m['idim'], m['odim']
        prev_s = META[t - 1]['s'] if t > 0 else S0
        halve = prev_s // s
        for g in range(NDIR):
            # replicate vertically so the q-split lhsT slice shares the rhs
            # partition base (PE requires equal base partitions)
            wl = _blockdiag(Weff[t][g][:idim], s)
            wr = _blockdiag(Weff[t][g][idim:], s)
            if idim <= 128 and halve > 1:
                wl = np.vstack([wl] * halve)
                wr = np.vstack([wr] * halve)
            add_const(f'w{t}_{g}_l', wl)
            add_const(f'w{t}_{g}_r', wr)
        if m['sd'] is not None:
            p = levels[t]
            sd = m['sd']
            src_s = META[t - SL]['s'] if t - SL >= 0 else S0
            if sd <= 128:
                # per bo-run zero-padded full-height weight: block sits at the
                # source slot-block rows, zeros elsewhere (PE matmul base
                # partition must be 0/32/64, so operands start at 0)
                gp = min(src_s * sd, 128) if t - SL >= 0 else S0 * 8
                wsb = _blockdiag(_np(p['Ws']).astype(np.float32), s)
                for ri, base in enumerate(skip_bases(t)):
                    wz = np.zeros((gp, wsb.shape[1]), np.float32)
                    wz[base * sd: base * sd + wsb.shape[0]] = wsb
                    add_const(f'ws{t}_{ri}', wz)
            else:
                add_const(f'ws{t}_0', _blockdiag(_np(p['Ws']).astype(np.float32), s))
            add_const(f'wmsA{t}', _blockdiag(_np(p['Wms']).astype(np.float32)[:odim], s))
            add_const(f'wmsB{t}', _blockdiag(_np(p['Wms']).astype(np.float32)[odim:], s))
            flat, src_och, src_cols = build_skip_idx(t, perms, tree)
            cols_t, och_t = m['cols'], m['och']
            CH = min(512 // och_t, cols_t)
            blocks = []
            for c0 in range(0, cols_t, CH):
                cw = min(CH, cols_t - c0)
                chunk = np.concatenate([flat[c0:c0 + cw] + oc * src_cols
                                        for oc in range(src_och)])
                # wrapped: out column i of a 16-partition group reads
                # idx[i % 16, i // 16]
                blocks.append(chunk.reshape(-1, 16).T)      # [16, L/16]
            wrapped = np.concatenate(blocks, axis=1)
            add_const(f'sidx{t}', np.tile(wrapped, (8, 1)).astype(np.uint16))

    with tile.TileContext(nc) as tc:
        with tc.tile_pool(name='sb', bufs=1) as sb, \
             tc.tile_pool(name='ps', bufs=1, space='PSUM') as ps:
            _emit(tc, nc, sb, ps, ext, alphas, la, y_out, mybir, dbg)
    nc.compile()
    return nc, wconst


def _emit(tc, nc, sb, ps, ext, alphas, la, y_out, mybir, dbg={}):
    f32 = mybir.dt.float32
    AF = mybir.ActivationFunctionType

    # resident weights into SBUF (deep-level weights are streamed per use)
    W = {}
    for name, h in ext.items():
        if name == 'leaf_rhs' or name.startswith('mask') \
                or name.startswith('sidx') or _is_streamed(name):
            continue
        tl = sb.tile(list(h.shape), h.dtype, tag=name, name=f'w_{name}')
        nc.sync.dma_start(tl[:], h[:])
        W[name] = tl

    S = {}
    S[-1] = sb.tile([S0 * 8, LEAF_COLS], f32, tag='Sr0', name='S_leaf')
    for t in range(NL):
        S[t] = sb.tile([128, META[t]['och'] * META[t]['cols']], f32,
                       tag=f'Sr{(t + 1) % 4}', name=f'S{t}')

    # ---- leaf ----
    lrhs = sb.tile([S0 * 3, LEAF_COLS], f32, tag='Sr1', name='leaf_rhs_t')
    nc.sync.dma_start(lrhs[:], ext['leaf_rhs'][:])
    for c0 in range(0, LEAF_COLS, 512):
        cw = min(512, LEAF_COLS - c0)
        pt = ps.tile([S0 * 8, cw], f32, tag='bank0', name=f'leaf_ps_{c0}')
        nc.tensor.matmul(pt[:], W['lw_bd'][:], lrhs[:, c0:c0 + cw],
                         start=True, stop=True)
        nc.scalar.activation(S[-1][:, c0:c0 + cw], pt[:], AF.Prelu, alpha=la)

    if -1 in dbg:
        nc.sync.dma_start(dbg[-1][:], S[-1][:])

    # ---- levels ----
    for t in range(NL):
        _emit_level(nc, sb, ps, ext, W, S, t, alphas[t], mybir)
        if t in dbg:
            nc.sync.dma_start(dbg[t][:], S[t][:])

    # ---- output ----
    m = META[NL - 1]
    och, cols = m['och'], m['cols']          # 4, 32
    dst = y_out[:].rearrange('b (oc p) -> oc p b', oc=och, p=128)
    for oc in range(och):
        nc.sync.dma_start(dst[oc], S[NL - 1][:, oc * cols:(oc + 1) * cols])


def _emit_level(nc, sb, ps, ext, W, S, t, al, mybir):
    f32 = mybir.dt.float32
    AF = mybir.ActivationFunctionType
    m = META[t]
    s, nb, M, idim, odim, sd = m['s'], m['nb'], m['M'], m['idim'], m['odim'], m['sd']
    och, cols = m['och'], m['cols']
    if t > 0:
        pm = META[t - 1]
        ps_, pnb, pM, podim, poch, pcols = pm['s'], pm['nb'], pm['M'], \
            min(pm['odim'], 128), pm['och'], pm['cols']
    else:
        ps_, pnb, pM, podim, poch, pcols = S0, NB0, N, 8, 1, LEAF_COLS
    halve = ps_ // s
    Sin, Sout = S[t - 1], S[t]
    kch = max(1, idim // 128)            # contract chunks when idim > 128
    # K rows per child-matmul-chunk: s*idim/kch ; source features per chunk:
    # for kch>1 (s==1) chunk k covers source oc=k block.
    CH = 512 // och                       # columns per chunk
    CH = min(CH, cols)

    for c0 in range(0, cols, CH):
        cw = min(CH, cols - c0)
        # ---- masks for this chunk ----
        mk = sb.tile([128, 5, och * cw], mybir.dt.int8, tag=f'mk{t % 2}', name=f'mk_{t}_{c0}')
        for g in range(5):
            for oc in range(och):
                nc.sync.dma_start(
                    mk[:, g, oc * cw:(oc + 1) * cw],
                    ext[f'mask{t}'][g, :, oc * cols + c0: oc * cols + c0 + cw])

        # ---- 6 expert banks ----
        banks = []
        for g in range(NDIR):
            bk = ps.tile([128, och * cw], f32, tag=f'bank{g}', name=f'bank{g}_{t}_{c0}')
            banks.append(bk)
            for oc in range(och):
                started = False
                for q in range(halve):
                    lo = max(c0, q * pnb * M)
                    hi = min(c0 + cw, (q + 1) * pnb * M)
                    if lo >= hi:
                        continue
                    for eps in range(2):
                        wt = W.get(f'w{t}_{g}_' + ('lr'[eps]))
                        for k in range(kch):
                            rhs = _rhs_ap(Sin, lo, hi, q, k, eps, s, idim, podim,
                                          pM, pnb, M, pcols, kch)
                            wname = f'w{t}_{g}_' + ('lr'[eps])
                            if wname in W:
                                if kch == 1:
                                    mcols = s * odim
                                    clo = oc * 128 if och > 1 else 0
                                    ccnt = 128 if och > 1 else mcols
                                    lhs = _ap(wt[:], q * s * podim, s * idim,
                                              [[1, ccnt]], clo)
                                else:
                                    lhs = _lhs_ap(wt, k, kch, oc, och, s, idim, odim)
                            else:
                                assert halve == 1
                                lhs = _get_lhs(nc, sb, ext, W, mybir, wname, k,
                                               kch, oc, och, s, idim, odim,
                                               f'{t}_{g}_{eps}_{k}_{oc}_{c0}')
                            nc.tensor.matmul(
                                banks[g][:, oc * cw + (lo - c0): oc * cw + (hi - c0)],
                                lhs, rhs, start=(eps == 0 and k == 0),
                                stop=(eps == 1 and k == kch - 1))

        # ---- select: acc = bank[v] ----
        acc = sb.tile([128, och * cw], f32, tag=f'acc{t % 2}', name=f'acc_{t}_{c0}')
        nc.scalar.copy(acc[:], banks[0][:])
        for g in range(1, NDIR):
            nc.vector.copy_predicated(acc[:], mk[:, g - 1, :], banks[g][:])
        if sd is None:
            nc.scalar.activation(Sout[:, _csl(c0, cw, och, cols)], acc[:],
                                 AF.Prelu, alpha=al['am'])
            continue

        # ---- sampled layer ----
        main = sb.tile([128, och * cw], f32, tag=f'main{t % 2}', name=f'main_{t}_{c0}')
        nc.scalar.activation(main[:], acc[:], AF.Prelu, alpha=al['am'])

        ts = t - SL
        if ts >= 0:
            sm_ = META[ts]
            src_s, src_nb, src_och, src_cols = sm_['s'], sm_['nb'], sm_['och'], sm_['cols']
        else:
            src_s, src_nb, src_och, src_cols = S0, NB0, 1, LEAF_COLS
        gp = 128
        Ssrc = S[ts]
        # gather: gath[p, (oc_src, c)] = Ssrc[p, sidx-wrapped[...]]
        nvi = src_och * cw
        gath = sb.tile([gp, nvi], f32, tag=f'gath{t % 2}', name=f'gath_{t}_{c0}')
        six = sb.tile([gp, nvi // 16], mybir.dt.uint16, tag=f'six{t % 2}',
                      name=f'six_{t}_{c0}')
        blk = (c0 // CH) * nvi // 16 if och == 1 else 0
        nc.sync.dma_start(six[:], ext[f'sidx{t}'][:, blk: blk + nvi // 16])
        nc.gpsimd.indirect_copy(gath[:], Ssrc[:], six[:],
                                i_know_ap_gather_is_preferred=True)

        # smp matmul: per bo-run split, K = s*sd partitions of gath
        bases = skip_bases(t)
        smp_ps = ps.tile([128, och * cw], f32, tag='aux0', name=f'smp_ps_{t}_{c0}')
        for oc in range(och):
            for ri, base in enumerate(bases):
                lo = max(c0, ri * src_nb * M)
                hi = min(c0 + cw, (ri + 1) * src_nb * M)
                if lo >= hi:
                    continue
                skch = max(1, sd // 128)
                for k in range(skch):
                    # full-height K (zeros off-block); for skch>1 (s==1,
                    # sd>128): feature chunk k lives in gath's oc_src=k block
                    plo = 0
                    pcnt = gp if skch == 1 else 128
                    fdims, foff = _gfree(lo, hi, cw, c0, k, skch, M)
                    rhs = _ap(gath[:], plo, pcnt, fdims, foff)
                    wname = f'ws{t}_{ri if skch == 1 else 0}'
                    if wname in W:
                        wt = W[wname]
                        if skch == 1:
                            mcols = s * odim
                            clo = oc * 128 if och > 1 else 0
                            ccnt = 128 if och > 1 else mcols
                            lhs = _ap(wt[:], 0, pcnt, [[1, ccnt]], clo)
                        else:
                            lhs = _lhs_ap(wt, k, skch, oc, och, s, sd, odim)
                    else:
                        kd = sd if skch > 1 else gp // s
                        lhs = _get_lhs(nc, sb, ext, W, mybir, wname, k,
                                       skch, oc, och, s, kd, odim,
                                       f's{t}_{ri}_{k}_{oc}_{c0}')
                    nc.tensor.matmul(
                        smp_ps[:, oc * cw + (lo - c0): oc * cw + (hi - c0)],
                        lhs, rhs, start=(k == 0), stop=(k == skch - 1))
        smp = sb.tile([128, och * cw], f32, tag=f'smp{t % 2}', name=f'smp_{t}_{c0}')
        nc.scalar.activation(smp[:], smp_ps[:], AF.Prelu, alpha=al['als'])

        # out2 = main @ WmsA + smp @ WmsB
        out2 = ps.tile([128, och * cw], f32, tag='aux1', name=f'out2_{t}_{c0}')
        ikch = max(1, odim // 128)
        for oc in range(och):
            mms = [(buf, wname, k) for (buf, wname) in
                   ((main, f'wmsA{t}'), (smp, f'wmsB{t}')) for k in range(ikch)]
            for i, (buf, wname, k) in enumerate(mms):
                rhs = _ap(buf[:], 0, 128, [[1, cw]], k * cw) if ikch > 1 \
                    else buf[:]
                if wname in W:
                    lhs = _lhs_ap(W[wname], k, ikch, oc, och, s, odim, odim)
                else:
                    lhs = _get_lhs(nc, sb, ext, W, mybir, wname, k, ikch, oc,
                                   och, s, odim, odim, f'm{t}_{i}_{oc}_{c0}')
                nc.tensor.matmul(out2[:, oc * cw:(oc + 1) * cw], lhs, rhs,
                                 start=(i == 0), stop=(i == len(mms) - 1))
        nc.scalar.activation(Sout[:, _csl(c0, cw, och, cols)], out2[:],
                             AF.Prelu, alpha=al['alms'])


def _is_streamed(name):
    """Weights of deep levels are DMA-streamed per matmul instead of resident."""
    for t in range(8, NL):
        if name.startswith(f'w{t}_'):
            return True
    for t in range(9, NL):
        if name in (f'wmsA{t}', f'wmsB{t}') or name.startswith(f'ws{t}_'):
            return True
    return False


def _get_lhs(nc, sb, ext, W, mybir, name, k, kch, oc, och, s, kdim, odim, uid):
    """lhsT [K-chunk rows, M-chunk cols] either from the resident tile or as a
    freshly streamed piece. Handles the partition-folded (>128 rows) storage."""
    rows = s * kdim
    mcols = s * odim
    clo = oc * 128 if och > 1 else 0
    ccnt = 128 if och > 1 else mcols
    if name in W:
        return _lhs_ap(W[name], k, kch, oc, och, s, kdim, odim)
    h = ext[name]
    if rows <= 128:
        rlo, rcnt = k * (rows // kch), rows // kch
        srcap = h[rlo:rlo + rcnt, clo:clo + ccnt]
    else:
        rcnt = 128
        srcap = h[0:128, k * mcols + clo: k * mcols + clo + ccnt]
    piece = sb.tile([rcnt, ccnt], mybir.dt.float32, tag='wstr', bufs=6,
                    name=f'wp_{name}_{uid}')
    nc.sync.dma_start(piece[:], srcap)
    return piece[:]


def _csl(c0, cw, och, cols):
    """Output slice helper: contiguous if och==1 else per-oc writes needed."""
    if och == 1:
        return slice(c0, c0 + cw)
    # for och>1 the level has a single chunk (cols <= CH), so c0==0, cw==cols
    assert c0 == 0 and cw == cols
    return slice(0, och * cols)


def _rhs_ap(Sin, lo, hi, q, k, eps, s, idim, podim, pM, pnb, M, pcols, kch):
    """rhs [K, hi-lo] on Sin for target cols [lo, hi) of split q, child eps,
    contract chunk k."""
    if kch == 1:
        # source partitions [q*s*podim + ...] ; idim == podim here
        plo = q * s * podim
        pcnt = s * idim
        foff_oc = 0
    else:
        # s == 1, idim > 128: chunk k = source oc block k (128 feats)
        plo = 0
        pcnt = 128
        foff_oc = k * pcols
    bo_lo = lo // M
    jlo = lo % M
    ncols = hi - lo
    if ncols <= M - jlo:
        fdims = [[2, ncols]]
        foff = (bo_lo % pnb) * pM + 2 * jlo + eps
    else:
        assert jlo == 0 and ncols % M == 0
        fdims = [[pM, ncols // M], [2, M]]
        foff = (bo_lo % pnb) * pM + eps
    return _ap(Sin[:], plo, pcnt, fdims, foff_oc + foff)


def _gfree(lo, hi, cw, c0, k, skch, M):
    """free dims for gathered skip tensor [128, src_och*cw] ordered (oc_src, c)."""
    foff = k * cw + (lo - c0)
    return [[1, hi - lo]], foff


def _lhs_ap(wt, k, kch, oc, och, s, kdim, odim):
    """lhsT slice [K-chunk rows, M(=oc chunk of s*odim)]. Weights with more
    than 128 rows are stored partition-folded: [128, kch*mcols], chunk k at
    free offset k*mcols."""
    full = wt[:]
    rows = s * kdim
    mcols = s * odim
    clo = oc * 128 if och > 1 else 0
    ccnt = 128 if och > 1 else mcols
    if rows <= 128:
        if kch == 1 and och == 1:
            return full
        rlo = k * (rows // kch)
        rcnt = rows // kch
        return _ap(full, rlo, rcnt, [[1, ccnt]], clo)
    assert kch == rows // 128
    return _ap(full, 0, 128, [[1, ccnt]], k * mcols + clo)


# ---------------------------------------------------------------------------
# run
# ---------------------------------------------------------------------------

_CACHE = {}
PROFILE = False
LAST_EXEC_NS = None
LAST_TRACE = None


def _run_device(points, vecs, tree, perms, lw, la, Weff, alphas, levels):
    global LAST_EXEC_NS, LAST_TRACE
    from concourse.bass_utils import run_bass_kernel_spmd
    key = 'prog'
    if key not in _CACHE:
        _CACHE[key] = build_program(Weff, alphas, levels, tree, perms, lw, la)
    nc, wconst = _CACHE[key]
    in_maps = []
    for core in range(NCORES):
        arrs = build_core_arrays(core, points, vecs, perms)
        arrs.update(wconst)
        in_maps.append(arrs)
    kw = {}
    if PROFILE:
        kw = dict(trace=True, trace_cores=[0])
    res = run_bass_kernel_spmd(nc, in_maps, core_ids=list(range(NCORES)), **kw)
    if PROFILE:
        LAST_EXEC_NS = res.exec_time_ns
        LAST_TRACE = res.instructions_and_trace[1] if res.instructions_and_trace else None
    return np.concatenate([r['y'] for r in res.results], axis=0)
